# revision 21
# baseline (speedup 1.0000x reference)
"""Circulant 1x1 conv (nn_Circulant1x1Conv) as a Trainium2 Bass kernel.

Math: the reference does, per spatial position r (N = batch*h*w rows):
    y[r, s*C + n] = irfft(rfft(x[r, :]) * cf[s])[n]  (circular convolution)
which is exactly a matmul  Y(N, 2048) = X(N, 512) @ W(512, 2048)  with
    W[k, s*C + n] = c_s[(n - k) mod C],   c_s = irfft(cf[s], n=C).

Crucially the native memory layouts are already transposed the right way:
  x[b] viewed as (C=512, h*w=1024) is X^T for that batch, and the output
  (nstack*C=2048, h*w) per batch is Y^T. So per batch:
      Out_b (2048, hw) = W^T @ X_b  ==  matmul(out, lhsT=W, rhs=X_b)
  on the tensor engine with zero data transposes anywhere.

Sharding: data-parallel over batch, 4 batches per core x 8 cores. Each core
computes a (2048, 4096) = (512, 2048)^T @ (512, 4096) matmul.

Precision knob DT_KIND:
  - "f32r": fp32 data, PE in fp32r (replicated/TF32-like) mode: 1 cycle/row
            at free-dim >= 256 per the cost model -> bf16-speed w/ fp32 inputs.
  - "bf16": inputs cast to bf16 on host; ~5e-3 rel error.
  - "f32":  exact fp32 matmul, 4 cycles/row (slow; debugging fallback).
"""

import numpy as np

SIZE = 512          # channels C (circulant size)
NSTACK = 4
BATCH = 32
HW = 32 * 32
N_CORES = 8
BPC = BATCH // N_CORES          # batches per core = 4
COLS = BPC * HW                 # moving free dim per core = 4096
M_OUT = NSTACK * SIZE           # output channels = 2048
P = 128
KC = SIZE // P                  # contraction chunks = 4
MT = M_OUT // P                 # output row tiles = 16
NFREE = 512                     # matmul moving free dim (1 PSUM bank fp32)
NT = COLS // NFREE              # moving chunks = 8
GN = 4                          # psum tiles per group (half of PSUM banks)
NG = NT // GN                   # groups per m-tile = 2

DT_KIND = "f32r"

_CACHE = {}


def _build_nc(dt_kind):
    import concourse.bacc as bacc
    import concourse.tile as tile
    from concourse import mybir

    io_dt = {"bf16": mybir.dt.bfloat16,
             "f32r": mybir.dt.float32r,
             "f32": mybir.dt.float32}[dt_kind]

    nc = bacc.Bacc("TRN2", name="circulant1x1")
    x = nc.dram_tensor("x", [SIZE, COLS], io_dt, kind="ExternalInput")
    w = nc.dram_tensor("w", [SIZE, M_OUT], io_dt, kind="ExternalInput")
    out = nc.dram_tensor("out", [M_OUT, COLS], mybir.dt.float32,
                         kind="ExternalOutput")

    with tile.TileContext(nc) as tc:
        with (
            tc.tile_pool(name="xin", bufs=1) as xp,
            tc.tile_pool(name="win", bufs=1) as wp,
            tc.tile_pool(name="outp", bufs=8) as op,
            tc.tile_pool(name="outpt", bufs=2) as opt,
            tc.tile_pool(name="ps", bufs=8, space="PSUM") as pp,
        ):
            HCOL = COLS // NG                   # columns per group = 2048
            x_sb = xp.tile([P, KC, COLS], io_dt)
            w_sb = wp.tile([P, KC, M_OUT], io_dt)

            # Input DMAs on the Sync HWDGE queue (outputs go on Scalar's),
            # ordered so the PE can start as early as possible and is never
            # gated on bytes it doesn't need yet: first the m0..m3 weight
            # columns (warmup fodder + ramp weights, 1 MB), then all of x's
            # group-0 half (the ramp tracks these arrivals and m1..m3
            # sweeps run dep-free on them), then the remaining weight
            # columns, then x's group-1 half.
            WR = 4 * P                          # ramp weight columns
            # k0's ramp columns go first as a small separate piece so the
            # PE warmup (which reads them) can start ~2us earlier.
            nc.sync.dma_start(out=w_sb[:, 0, 0:WR], in_=w[0:P, 0:WR])
            nc.sync.dma_start(
                out=w_sb[:, 1:, 0:WR],
                in_=w[P:, 0:WR].rearrange("(k p) c -> p k c", p=P))
            for k in range(KC):
                nc.sync.dma_start(out=x_sb[:, k, 0:HCOL],
                                  in_=x[k * P:(k + 1) * P, 0:HCOL])
            for k in range(KC):
                nc.sync.dma_start(out=w_sb[:, k, WR:M_OUT],
                                  in_=w[k * P:(k + 1) * P, WR:M_OUT])
            for k in range(KC):
                nc.sync.dma_start(out=x_sb[:, k, HCOL:COLS],
                                  in_=x[k * P:(k + 1) * P, HCOL:COLS])

            # HAM warmup: dummy matmuls on a zeroed scratch tile while the
            # inputs stream in, so the PE hits K=8/8 (2.4 GHz) before the
            # real matmuls begin. The memset (not a DMA) means warmup
            # starts right after the framework preamble. Results discarded.
            scratch0 = xp.tile([P, NFREE], mybir.dt.float32, tag="scratch0")
            scratch = xp.tile([P, NFREE], io_dt, tag="scratch")
            nc.vector.memset(scratch0[:], 0)
            nc.vector.tensor_copy(out=scratch[:], in_=scratch0[:])
            for i in range(10):
                wps = pp.tile([P, NFREE], mybir.dt.float32, tag="ps",
                              name=f"warm_{i}")
                nc.tensor.matmul(wps, scratch[:, 0:P], scratch[:, 0:NFREE],
                                 start=True, stop=True)

            def copy_out(j, dst, src):
                if j % 2 == 0:
                    nc.vector.tensor_copy(out=dst, in_=src)
                else:
                    nc.scalar.copy(out=dst, in_=src)

            def group_mms(m, g, ps, k):
                for j in range(GN):
                    col = (g * GN + j) * NFREE
                    nc.tensor.matmul(ps[j], w_sb[:, k, m * P:(m + 1) * P],
                                     x_sb[:, k, col:col + NFREE],
                                     start=(k == 0), stop=(k == KC - 1))

            def group_finish(m, g, ps):
                o_sb = op.tile([P, HCOL], mybir.dt.float32, tag="osb",
                               name=f"osb_{m}_{g}")
                for j in range(GN):
                    copy_out(j, o_sb[:, j * NFREE:(j + 1) * NFREE], ps[j])
                nc.sync.dma_start(
                    out=out[m * P:(m + 1) * P, g * HCOL:(g + 1) * HCOL],
                    in_=o_sb[:])

            def alloc_ps(m, g):
                return [pp.tile([P, NFREE], mybir.dt.float32, tag="ps",
                                name=f"ps_{m}_{g}_{j}") for j in range(GN)]

            # Ramp: m0/m1 group-0 blocks k-outer across all 8 PSUM banks,
            # tracking the x group-0 chunks as they land (8 matmuls per
            # chunk) so the PE never idles past the HAM re-throttle window.
            ps_r = [alloc_ps(0, 0), alloc_ps(1, 0)]
            for k in range(KC):
                for mi in range(2):
                    group_mms(mi, 0, ps_r[mi], k)
            for mi in range(2):
                group_finish(mi, 0, ps_r[mi])

            # Column-major sweeps: the rest of group 0 (m1..m3 dep-free on
            # the ramp-phase bytes, m4+ on the weight remainder that lands
            # behind them), then all of group 1.
            def sweep(m, g):
                ps = alloc_ps(m, g)
                for j in range(GN):
                    col = (g * GN + j) * NFREE
                    for k in range(KC):
                        nc.tensor.matmul(ps[j], w_sb[:, k, m * P:(m + 1) * P],
                                         x_sb[:, k, col:col + NFREE],
                                         start=(k == 0), stop=(k == KC - 1))
                if m == MT - 1 and g == 1:
                    # last group: split the staging/DMA in half so the
                    # kernel tail is one 512 KB DMA, not 1 MB behind 4
                    # serial copies.
                    for h in range(2):
                        o_h = opt.tile([P, HCOL // 2], mybir.dt.float32,
                                       tag="osbt", name=f"osbt_{h}")
                        for j2 in range(2):
                            copy_out(j2 + h, o_h[:, j2 * NFREE:(j2 + 1) * NFREE],
                                     ps[h * 2 + j2])
                        col0 = g * HCOL + h * (HCOL // 2)
                        nc.sync.dma_start(
                            out=out[m * P:(m + 1) * P, col0:col0 + HCOL // 2],
                            in_=o_h[:])
                else:
                    group_finish(m, g, ps)

            for m in range(2, MT):
                sweep(m, 0)
            for m in range(MT):
                sweep(m, 1)
    nc.compile()
    return nc


def get_nc(dt_kind=DT_KIND):
    if dt_kind not in _CACHE:
        _CACHE[dt_kind] = _build_nc(dt_kind)
    return _CACHE[dt_kind]


def build_weight(c_f):
    """(NSTACK, SIZE//2+1, 2) rfft coeffs -> circulant weight W (SIZE, M_OUT),
    W[k, s*SIZE + n] = c_s[(n - k) mod SIZE]."""
    c_f = np.asarray(c_f, np.float32)
    cf = c_f[..., 0].astype(np.float64) + 1j * c_f[..., 1].astype(np.float64)
    c = np.fft.irfft(cf, n=SIZE, axis=-1)            # (NSTACK, SIZE) float64
    idx = (np.arange(SIZE)[None, :] - np.arange(SIZE)[:, None]) % SIZE
    W = np.empty((SIZE, M_OUT), np.float32)
    for s in range(NSTACK):
        W[:, s * SIZE:(s + 1) * SIZE] = c[s][idx]
    return W


def _round_fp32r(a):
    """RNE-round fp32 to the fp32r storage format (e8m11 in the high 20
    bits of the word) — what the PE consumes in fp32r matmul mode."""
    u = np.ascontiguousarray(a, np.float32).view(np.uint32).copy()
    u += 0x7FF + ((u >> 12) & 1)
    u &= 0xFFFFF000
    return u.view(np.float32)


def make_in_maps(x, c_f, dt_kind=DT_KIND):
    x = np.asarray(x, np.float32)
    W = build_weight(c_f)
    if dt_kind == "bf16":
        import ml_dtypes
        cast = lambda a: np.ascontiguousarray(a).astype(ml_dtypes.bfloat16)
    elif dt_kind == "f32r":
        cast = _round_fp32r
    else:
        cast = lambda a: np.ascontiguousarray(a, np.float32)
    Wc = cast(W)
    in_maps = []
    for i in range(N_CORES):
        xs = (x[i * BPC:(i + 1) * BPC]
              .reshape(BPC, SIZE, HW)
              .transpose(1, 0, 2)
              .reshape(SIZE, COLS))
        in_maps.append({"x": cast(xs), "w": Wc})
    return in_maps


def assemble_output(per_core_outs):
    """list of 8 (M_OUT, COLS) fp32 -> (BATCH, M_OUT, 32, 32) fp32"""
    parts = [o.reshape(M_OUT, BPC, HW).transpose(1, 0, 2)
             for o in per_core_outs]
    out = np.concatenate(parts, axis=0)               # (BATCH, M_OUT, HW)
    return np.ascontiguousarray(out.reshape(BATCH, M_OUT, 32, 32), np.float32)


def run(x, c_f, dt_kind=DT_KIND, **run_kwargs):
    """Returns (full_output, BassKernelResults)."""
    from concourse.bass_utils import run_bass_kernel_spmd
    nc = get_nc(dt_kind)
    in_maps = make_in_maps(x, c_f, dt_kind)
    res = run_bass_kernel_spmd(nc, in_maps, core_ids=list(range(N_CORES)),
                               **run_kwargs)
    out = assemble_output([r["out"] for r in res.results])
    return out, res


def kernel(input, c_f):
    out, _ = run(input, c_f)
    return out
